# revision 1
# baseline (speedup 1.0000x reference)
"""GRU decoder (nn_Decoder) Trainium2 Bass kernel.

Strategy: pure data parallelism — batch B=8192 sharded over 8 NeuronCores
(1024 rows each), all weights replicated. On-device layout keeps features on
the partition axis and batch on the free axis (h.T is [H, B_c]), so the GRU
recurrence matmuls are stationary-weight PE matmuls streaming the batch.

Per core:
  - 3-layer MLP (fp32 PE matmuls) turns latent.T into the initial hidden
    state h0.T, stored bf16.
  - 65 fully unrolled GRU steps. Input-side gate pre-activations come from a
    one-hot matmul: gi_vocab = embed @ W_ih.T is only [32, 768], and the
    one-hot of the token (built host-side, with an extra constant-1 row that
    folds the input biases into the same matmul as a K=33 contraction) selects
    rows on the PE. Recurrent side is W_hh.T matmuls against h.T (bf16).
    r/z gates: single sigmoid ACT op over a 4-bank PSUM tile (biases already
    folded in). n gate: scalar_tensor_tensor fuses (h_n + b_hh_n) * r, an
    identity matmul accumulates it onto the i_n PSUM bank, tanh reads PSUM.
    h_new = n + z*(h-n) on the DVE (bf16, 2x mode).
  - Per step, the two output projections run on the PE and the result is
    DMA'd straight into the [B_c, T, A] output slice.
"""

import numpy as np
import ml_dtypes

B, L, H, A, T, E = 8192, 128, 256, 32, 65, 8
NCORES = 8
BC = B // NCORES          # 1024 batch rows per core
NCH = 2                   # batch chunks per step (free dim 512 each)
CH = BC // NCH            # 512
G3 = 3 * H                # 768

BF16 = ml_dtypes.bfloat16

_CACHE = {}


def _build(trace=False):
    """Build + finalize the bass module. Returns (nc, meta)."""
    import concourse.bass as bass
    import concourse.bacc as bacc
    import concourse.tile as tile
    from concourse import mybir
    from contextlib import ExitStack

    f32 = mybir.dt.float32
    bf16 = mybir.dt.bfloat16
    Alu = mybir.AluOpType
    Act = mybir.ActivationFunctionType

    nc = bacc.Bacc("TRN2", target_bir_lowering=False, debug=False)

    lat = nc.dram_tensor("lat", [128, BC], f32, kind="ExternalInput")
    oh = nc.dram_tensor("oh", [T, A + 1, BC], bf16, kind="ExternalInput")
    whh = nc.dram_tensor("whh", [128, 2, G3], bf16, kind="ExternalInput")
    giv = nc.dram_tensor("giv", [A + 1, G3], bf16, kind="ExternalInput")
    wd0 = nc.dram_tensor("wd0", [128, H], f32, kind="ExternalInput")
    wd1 = nc.dram_tensor("wd1", [128, 2, H], f32, kind="ExternalInput")
    wd2 = nc.dram_tensor("wd2", [128, 2, H], f32, kind="ExternalInput")
    wp1 = nc.dram_tensor("wp1", [128, 2, A], bf16, kind="ExternalInput")
    wp2 = nc.dram_tensor("wp2", [128, A], bf16, kind="ExternalInput")
    bias = nc.dram_tensor("bias", [128, 9], f32, kind="ExternalInput")
    bp2b = nc.dram_tensor("bp2b", [1, 4 * A], f32, kind="ExternalInput")
    u16 = mybir.dt.uint16
    tokw = nc.dram_tensor("tokw", [128, T, 64], u16, kind="ExternalInput")
    gtab = nc.dram_tensor("gtab", [128, 2, A], bf16, kind="ExternalInput")
    out = nc.dram_tensor("out", [BC, T, A], f32, kind="ExternalOutput")

    outv = out.rearrange("(c j p) t a -> c p j t a", c=NCH, j=4, p=128)

    with ExitStack() as ctx:
        tc = ctx.enter_context(tile.TileContext(nc))
        const = ctx.enter_context(tc.tile_pool(name="const", bufs=1))
        hp = ctx.enter_context(tc.tile_pool(name="hp", bufs=4))
        work = ctx.enter_context(tc.tile_pool(name="work", bufs=4))
        ohp = ctx.enter_context(tc.tile_pool(name="ohp", bufs=4))
        outp = ctx.enter_context(tc.tile_pool(name="outp", bufs=4))
        psum = ctx.enter_context(tc.tile_pool(name="psum", bufs=1, space="PSUM"))

        # ---- load constants ----
        lat_sb = const.tile([128, BC], f32, tag="lat")
        nc.sync.dma_start(out=lat_sb[:], in_=lat[:])
        whh_sb = const.tile([128, 2, G3], bf16, tag="whh")
        nc.sync.dma_start(out=whh_sb[:], in_=whh[:])
        giv_sb = const.tile([A + 1, G3], bf16, tag="giv")
        nc.sync.dma_start(out=giv_sb[:], in_=giv[:])
        wd0_sb = const.tile([128, H], f32, tag="wd0")
        nc.sync.dma_start(out=wd0_sb[:], in_=wd0[:])
        wd1_sb = const.tile([128, 2, H], f32, tag="wd1")
        nc.sync.dma_start(out=wd1_sb[:], in_=wd1[:])
        wd2_sb = const.tile([128, 2, H], f32, tag="wd2")
        nc.sync.dma_start(out=wd2_sb[:], in_=wd2[:])
        wp1_sb = const.tile([128, 2, A], bf16, tag="wp1")
        nc.sync.dma_start(out=wp1_sb[:], in_=wp1[:])
        wp2_sb = const.tile([128, A], bf16, tag="wp2")
        nc.sync.dma_start(out=wp2_sb[:], in_=wp2[:])
        bias_sb = const.tile([128, 9], f32, tag="bias")
        nc.sync.dma_start(out=bias_sb[:], in_=bias[:])
        bp2_sb = const.tile([128, 4 * A], f32, tag="bp2")
        nc.sync.dma_start(
            out=bp2_sb[:],
            in_=bass.AP(tensor=bp2b, offset=0, ap=[[0, 128], [1, 4 * A]]),
        )
        tokw_sb = const.tile([128, T, 64], u16, tag="tokw")
        nc.sync.dma_start(out=tokw_sb[:], in_=tokw[:])
        gtab_sb = const.tile([128, 2, A], bf16, tag="gtab")
        nc.sync.dma_start(out=gtab_sb[:], in_=gtab[:])

        # ---- MLP prologue: h0 = (relu(relu(lat@Wd0+b)@Wd1+b))@Wd2+b ----
        h1 = [work.tile([128, BC], f32, tag=f"mlp{m}", name=f"mlp{m}") for m in range(2)]
        for c in range(NCH):
            cs = slice(c * CH, (c + 1) * CH)
            ps = psum.tile([128, 2, CH], f32, tag="pr")
            for m in range(2):
                nc.tensor.matmul(
                    ps[:, m, :], wd0_sb[:, m * 128:(m + 1) * 128], lat_sb[:, cs],
                    start=True, stop=True,
                )
            for m in range(2):
                nc.vector.tensor_scalar(
                    out=h1[m][:, cs], in0=ps[:, m, :],
                    scalar1=bias_sb[:, 3:4] if m == 0 else bias_sb[:, 4:5],
                    scalar2=0.0, op0=Alu.add, op1=Alu.max,
                )
        h2 = [work.tile([128, BC], f32, tag=f"mlp2{m}", name=f"mlp2{m}") for m in range(2)]
        for c in range(NCH):
            cs = slice(c * CH, (c + 1) * CH)
            ps = psum.tile([128, 2, CH], f32, tag="phn", bufs=2)
            for m in range(2):
                for kc in range(2):
                    nc.tensor.matmul(
                        ps[:, m, :], wd1_sb[:, kc, m * 128:(m + 1) * 128],
                        h1[kc][:, cs], start=(kc == 0), stop=(kc == 1),
                    )
            for m in range(2):
                nc.vector.tensor_scalar(
                    out=h2[m][:, cs], in0=ps[:, m, :],
                    scalar1=bias_sb[:, 5:6] if m == 0 else bias_sb[:, 6:7],
                    scalar2=0.0, op0=Alu.add, op1=Alu.max,
                )
        h_cur = [hp.tile([128, BC], bf16, tag=f"h{m}", name=f"h0_{m}") for m in range(2)]
        for c in range(NCH):
            cs = slice(c * CH, (c + 1) * CH)
            ps = psum.tile([128, 2, CH], f32, tag="pz")
            for m in range(2):
                for kc in range(2):
                    nc.tensor.matmul(
                        ps[:, m, :], wd2_sb[:, kc, m * 128:(m + 1) * 128],
                        h2[kc][:, cs], start=(kc == 0), stop=(kc == 1),
                    )
            for m in range(2):
                nc.vector.tensor_scalar_add(
                    out=h_cur[m][:, cs], in0=ps[:, m, :],
                    scalar1=bias_sb[:, 7:8] if m == 0 else bias_sb[:, 8:9],
                )

        # ---- GRU steps ----
        PACKED_PROJ = False

        def emit_proj(h_tiles, tp):
            """Output projections for step tp (pipelined one step behind)."""
            for c in range(NCH):
                if PACKED_PROJ:
                    # p1 = relu(h @ Wp1 + bp1), packed: batch-subtile j ->
                    # col group j, so p1t is [4*32, 128]
                    p1ps = psum.tile([128, 128], f32, tag="phn", bufs=2,
                                     name=f"p1ps_{tp}_{c}")
                    for j in range(4):
                        bs = slice(c * CH + j * 128, c * CH + (j + 1) * 128)
                        for kc in range(2):
                            nc.tensor.matmul(
                                p1ps[32 * j:32 * (j + 1), :], wp1_sb[:, kc, :],
                                h_tiles[kc][:, bs],
                                start=(kc == 0), stop=(kc == 1),
                                tile_position=(0, 32 * j),
                            )
                    p1t = work.tile([128, 128], bf16, tag="p1t",
                                    name=f"p1t_{tp}_{c}")
                    nc.scalar.activation(
                        out=p1t[:], in_=p1ps[:], func=Act.Relu,
                        bias=bias_sb[:, 2:3],
                    )
                    # p2 = p1 @ Wp2 + bp2, row-group packed; out [128, 4*32]
                    p2ps = psum.tile([128, 4 * A], f32, tag="pz",
                                     name=f"p2ps_{tp}_{c}")
                    for j in range(4):
                        nc.tensor.matmul(
                            p2ps[:, j * A:(j + 1) * A],
                            p1t[32 * j:32 * (j + 1), :],
                            wp2_sb[32 * j:32 * (j + 1), :],
                            start=True, stop=True, tile_position=(32 * j, 0),
                        )
                else:
                    # simple: p1.T [32, 512] per chunk, p2 via 4 plain MMs
                    p1ps = psum.tile([A, CH], f32, tag="phn", bufs=2,
                                     name=f"p1ps_{tp}_{c}")
                    for kc in range(2):
                        nc.tensor.matmul(
                            p1ps[:], wp1_sb[:, kc, :],
                            h_tiles[kc][:, c * CH:(c + 1) * CH],
                            start=(kc == 0), stop=(kc == 1),
                        )
                    p1t = work.tile([A, CH], bf16, tag="p1t",
                                    name=f"p1t_{tp}_{c}")
                    nc.scalar.activation(
                        out=p1t[:], in_=p1ps[:], func=Act.Relu,
                        bias=bias_sb[0:A, 2:3],
                    )
                    p2ps = psum.tile([128, 4 * A], f32, tag="pz",
                                     name=f"p2ps_{tp}_{c}")
                    for j in range(4):
                        nc.tensor.matmul(
                            p2ps[:, j * A:(j + 1) * A],
                            p1t[:, j * 128:(j + 1) * 128], wp2_sb[0:A, :],
                            start=True, stop=True,
                        )
                outsb = outp.tile([128, 4 * A], f32, tag="outsb",
                                  name=f"outsb_{tp}_{c}")
                nc.vector.tensor_add(outsb[:], p2ps[:], bp2_sb[:])
                nc.sync.dma_start(
                    out=outv[c][:, :, tp, :],
                    in_=outsb.rearrange("p (j a) -> p j a", j=4),
                )

        h_prev = None
        for t in range(T):
            oh_t = ohp.tile([A + 1, BC], bf16, tag="oh", name=f"oh_{t}")
            nc.sync.dma_start(out=oh_t[:], in_=oh[t])
            # i_n + b_ih_n gathered by token on the (otherwise idle) GPSIMD
            gin = work.tile([128, 2, BC], bf16, tag="gin", name=f"gin_{t}")
            for m in range(2):
                nc.gpsimd.indirect_copy(
                    out=gin[:, m, :], data=gtab_sb[:, m, :],
                    idxs=tokw_sb[:, t, :],
                    i_know_ap_gather_is_preferred=True,
                )

            h_new = [hp.tile([128, BC], bf16, tag=f"h{m}", name=f"h_{t}_{m}")
                     for m in range(2)]
            for c in range(NCH):
                cs = slice(c * CH, (c + 1) * CH)
                pr = psum.tile([128, 2, CH], f32, tag="pr", name=f"pr_{t}_{c}")
                pz = psum.tile([128, 2, CH], f32, tag="pz", name=f"pz_{t}_{c}")
                phn = psum.tile([128, 2, CH], f32, tag="phn", bufs=2, name=f"phn_{t}_{c}")
                # one-hot matmuls first: they are always ready, so they
                # prefill the PSUM groups while PE waits for h_new
                for m in range(2):
                    nc.tensor.matmul(
                        pr[:, m, :], giv_sb[:, m * 128:(m + 1) * 128],
                        oh_t[:, cs], start=True, stop=False,
                    )
                for m in range(2):
                    nc.tensor.matmul(
                        pz[:, m, :], giv_sb[:, 256 + m * 128:256 + (m + 1) * 128],
                        oh_t[:, cs], start=True, stop=False,
                    )
                # recurrent matmuls: kc=0 first (h row-tile 0 is ready first),
                # chain-feeding targets (phn, pr) before pz
                for kc in range(2):
                    for m in range(2):
                        nc.tensor.matmul(
                            phn[:, m, :],
                            whh_sb[:, kc, 512 + m * 128:512 + (m + 1) * 128],
                            h_cur[kc][:, cs], start=(kc == 0), stop=(kc == 1),
                        )
                    for m in range(2):
                        nc.tensor.matmul(
                            pr[:, m, :], whh_sb[:, kc, m * 128:(m + 1) * 128],
                            h_cur[kc][:, cs], start=False, stop=(kc == 1),
                        )
                    for m in range(2):
                        nc.tensor.matmul(
                            pz[:, m, :],
                            whh_sb[:, kc, 256 + m * 128:256 + (m + 1) * 128],
                            h_cur[kc][:, cs], start=False, stop=(kc == 1),
                        )

                # per row-tile fast path: sigmoid(r_m) -> npre -> pin+= ->
                # tanh_m -> combine_m, so m=0 completes without waiting m=1
                rz = work.tile([128, 4, CH], bf16, tag="rz", name=f"rz_{t}_{c}")
                npre = work.tile([128, 2, CH], bf16, tag="npre", name=f"npre_{t}_{c}")
                t2v = work.tile([128, 2, CH], bf16, tag="t2v", name=f"t2v_{t}_{c}")
                nsb = work.tile([128, 2, CH], bf16, tag="nsb", name=f"nsb_{t}_{c}")
                t3 = work.tile([128, 2, CH], bf16, tag="t3", name=f"t3_{t}_{c}")
                for m in range(2):
                    nc.scalar.activation(
                        out=rz[:, m, :], in_=pr[:, m, :], func=Act.Sigmoid)
                    nc.vector.scalar_tensor_tensor(
                        out=npre[:, m, :], in0=phn[:, m, :],
                        scalar=bias_sb[:, m:m + 1], in1=rz[:, m, :],
                        op0=Alu.add, op1=Alu.mult,
                    )
                    if m == 0:
                        nc.vector.tensor_add(
                            t2v[:, m, :], npre[:, m, :], gin[:, m, cs])
                    else:
                        nc.gpsimd.tensor_add(
                            t2v[:, m, :], npre[:, m, :], gin[:, m, cs])
                # z = sigmoid(pz) (off the critical chain)
                nc.scalar.activation(out=rz[:, 2:4, :], in_=pz[:], func=Act.Sigmoid)
                for m in range(2):
                    nc.scalar.activation(
                        out=nsb[:, m, :], in_=t2v[:, m, :], func=Act.Tanh)
                    # h_new = n + z*(h - n)
                    nc.vector.tensor_sub(
                        t3[:, m, :], h_cur[m][:, cs], nsb[:, m, :])
                    nc.vector.tensor_mul(t3[:, m, :], rz[:, 2 + m, :], t3[:, m, :])
                    nc.vector.tensor_add(
                        h_new[m][:, cs], nsb[:, m, :], t3[:, m, :])

            if h_prev is not None:
                emit_proj(h_prev, t - 1)
            h_prev = h_new
            h_cur = h_new
        emit_proj(h_prev, T - 1)

    nc.finalize()
    return nc


def _prep_inputs(latent, target, embed, W_ih, b_ih, W_hh, b_hh,
                 Wd0, bd0, Wd1, bd1, Wd2, bd2, Wp1, bp1, Wp2, bp2):
    f32 = np.float32
    latent = np.asarray(latent, dtype=f32)
    target = np.asarray(target)
    embed = np.asarray(embed, dtype=f32)
    W_ih = np.asarray(W_ih, dtype=f32)
    b_ih = np.asarray(b_ih, dtype=f32)
    W_hh = np.asarray(W_hh, dtype=f32)
    b_hh = np.asarray(b_hh, dtype=f32)

    # one-hot tokens (teacher forcing shift), time-major, plus a const-1 row
    tokens = np.concatenate(
        [np.zeros((B, 1), dtype=np.int64), np.asarray(target[:, :-1], dtype=np.int64)],
        axis=1,
    )  # [B, T]
    ohf = np.zeros((T, A + 1, B), dtype=BF16)
    tok_tm = tokens.T  # [T, B]
    for a in range(A):
        ohf[:, a, :] = (tok_tm == a)
    ohf[:, A, :] = 1.0

    giv = embed @ W_ih.T  # [A, 3H]
    brow = np.empty((G3,), dtype=f32)
    brow[: 2 * H] = (b_ih + b_hh)[: 2 * H]
    brow[2 * H:] = b_ih[2 * H:]
    giv_aug = np.concatenate([giv, brow[None, :]], axis=0).astype(BF16)  # [33, 768]

    whhT = np.ascontiguousarray(W_hh.T)  # [H, 3H]
    whh_l = np.ascontiguousarray(
        whhT.reshape(2, 128, G3).transpose(1, 0, 2)
    ).astype(BF16)  # [128, 2, 768]

    wd0_l = np.ascontiguousarray(np.asarray(Wd0, dtype=f32))          # [128, 256]
    wd1_l = np.ascontiguousarray(
        np.asarray(Wd1, dtype=f32).reshape(2, 128, H).transpose(1, 0, 2))
    wd2_l = np.ascontiguousarray(
        np.asarray(Wd2, dtype=f32).reshape(2, 128, H).transpose(1, 0, 2))
    wp1_l = np.ascontiguousarray(
        np.asarray(Wp1, dtype=f32).reshape(2, 128, A).transpose(1, 0, 2)).astype(BF16)
    wp2_l = np.ascontiguousarray(
        np.tile(np.asarray(Wp2, dtype=f32), (4, 1))).astype(BF16)  # [128, 32]

    bias_pack = np.zeros((128, 9), dtype=f32)
    bias_pack[:, 0] = b_hh[2 * H: 2 * H + 128]
    bias_pack[:, 1] = b_hh[2 * H + 128:]
    bias_pack[:, 2] = np.tile(np.asarray(bp1, dtype=f32), 4)
    bias_pack[:, 3] = np.asarray(bd0, dtype=f32)[:128]
    bias_pack[:, 4] = np.asarray(bd0, dtype=f32)[128:]
    bias_pack[:, 5] = np.asarray(bd1, dtype=f32)[:128]
    bias_pack[:, 6] = np.asarray(bd1, dtype=f32)[128:]
    bias_pack[:, 7] = np.asarray(bd2, dtype=f32)[:128]
    bias_pack[:, 8] = np.asarray(bd2, dtype=f32)[128:]

    bp2b = np.ascontiguousarray(
        np.tile(np.asarray(bp2, dtype=f32), 4)[None, :])  # [1, 128]

    # n-gate input table for the GPSIMD gather: giv_n.T + b_ih_n, [128, 2, A]
    givT_n = giv.T[2 * H:] + b_ih[2 * H:, None]          # [256, 32]
    gtab = np.ascontiguousarray(
        givT_n.reshape(2, 128, A).transpose(1, 0, 2)).astype(BF16)

    latT = np.ascontiguousarray(latent.T)  # [128, B]

    shared = dict(whh=whh_l, giv=giv_aug, wd0=wd0_l, wd1=wd1_l, wd2=wd2_l,
                  wp1=wp1_l, wp2=wp2_l, bias=bias_pack, bp2b=bp2b, gtab=gtab)
    in_maps = []
    for c in range(NCORES):
        bs = slice(c * BC, (c + 1) * BC)
        m = dict(shared)
        m["lat"] = np.ascontiguousarray(latT[:, bs])
        m["oh"] = np.ascontiguousarray(ohf[:, :, bs])
        # tokens wrapped for indirect_copy: index i lives at partition
        # (i%16), column (i//16), replicated across the 8 Q7 core groups
        tok_c = tokens[bs]                               # [1024, T]
        w = tok_c.reshape(64, 16, T).transpose(1, 2, 0)  # [16, T, 64]
        m["tokw"] = np.ascontiguousarray(
            np.tile(w, (8, 1, 1))).astype(np.uint16)     # [128, T, 64]
        in_maps.append(m)
    return in_maps


def kernel(**inputs):
    from concourse.bass_utils import run_bass_kernel_spmd

    if "nc" not in _CACHE:
        _CACHE["nc"] = _build()
    nc = _CACHE["nc"]

    in_maps = _prep_inputs(**inputs)
    res = run_bass_kernel_spmd(nc, in_maps, core_ids=list(range(NCORES)))
    outs = [r["out"] for r in res.results]
    return np.concatenate(outs, axis=0).astype(np.float32)



# revision 2
# speedup vs baseline: 1.0200x; 1.0200x over previous
"""GRU decoder (nn_Decoder) Trainium2 Bass kernel, v2.

Data parallel: batch 8192 sharded over 8 cores (1024 rows each), weights
replicated. Per-core layout: features on partitions, batch on free axis.

Key structure per GRU step (vs v1 baseline):
  - Input-side r/z gate pre-acts via fp8e4 DoubleRow one-hot matmuls
    (K=34 incl const-1 bias row, 0.5 cyc/row) accumulated with the bf16
    recurrent matmuls into PSUM, everything scaled x16 so fp8 tables stay
    in the normal range; sigmoids apply scale=1/16 for free on ACT.
  - b_hh n-part bias folded into the pn PSUM bank via a fp8 bias-row
    matmul, so npre = pn * r is a plain tensor_tensor (no scalar op).
  - n-part input contribution gathered by GPSIMD (gtab includes b_ih and
    the x16 scale).
  - ACT instructions merged to [128,2,CH] granularity (6 per step).
  - GPSIMD does the p1 relu (+bp1) from PSUM; bp2 folded into the p2
    matmul via a const-1 row in p1t.
  - Engine balance: DVE does npre-c0, t2, combine, p2 copies; GPSIMD does
    gathers, relu, npre-c1.
"""

import numpy as np
import ml_dtypes

B, L, H, A, T, E = 8192, 128, 256, 32, 65, 8
NCORES = 8
BC = B // NCORES          # 1024 batch rows per core
NCH = 2                   # batch chunks per step
CH = BC // NCH            # 512
G3 = 3 * H                # 768
S = 16.0                  # gate pre-act scale (power of 2)
KOH = 17                  # one-hot DoubleRow k-tile partitions (2*17=34 rows)

BF16 = ml_dtypes.bfloat16
FP8 = ml_dtypes.float8_e4m3

_CACHE = {}


def _build(trace=False, nsteps=T, skip=(), opt=None):
    opt = opt or {}
    import concourse.bass as bass
    import concourse.bacc as bacc
    import concourse.tile as tile
    from concourse import mybir
    from contextlib import ExitStack

    f32 = mybir.dt.float32
    bf16 = mybir.dt.bfloat16
    fp8 = mybir.dt.float8e4
    u16 = mybir.dt.uint16
    Alu = mybir.AluOpType
    Act = mybir.ActivationFunctionType
    DR = mybir.MatmulPerfMode.DoubleRow

    nc = bacc.Bacc("TRN2", target_bir_lowering=False, debug=False)

    lat = nc.dram_tensor("lat", [128, BC], f32, kind="ExternalInput")
    oh = nc.dram_tensor("oh", [T, KOH, 2, BC], fp8, kind="ExternalInput")
    whh = nc.dram_tensor("whh", [128, 2, G3], bf16, kind="ExternalInput")
    trz = nc.dram_tensor("trz", [KOH, 2, 512], fp8, kind="ExternalInput")
    tnb = nc.dram_tensor("tnb", [KOH, 2, 256], fp8, kind="ExternalInput")
    ging = nc.dram_tensor("ging", [T, 128, 2, BC], bf16, kind="ExternalInput")
    wd0 = nc.dram_tensor("wd0", [128, H], f32, kind="ExternalInput")
    wd1 = nc.dram_tensor("wd1", [128, 2, H], f32, kind="ExternalInput")
    wd2 = nc.dram_tensor("wd2", [128, 2, H], f32, kind="ExternalInput")
    wp1 = nc.dram_tensor("wp1", [128, 2, A], bf16, kind="ExternalInput")
    wp2 = nc.dram_tensor("wp2", [128, A], bf16, kind="ExternalInput")
    bp2b = nc.dram_tensor("bp2b", [1, 4 * A], f32, kind="ExternalInput")
    bias = nc.dram_tensor("bias", [128, 9], f32, kind="ExternalInput")
    out = nc.dram_tensor("out", [BC, T, A], f32, kind="ExternalOutput")

    # out viewed per chunk: [c, 128-part, 4, T, A]
    outv = out.rearrange("(c j p) t a -> c p j t a", c=NCH, j=4, p=128)

    with ExitStack() as ctx:
        tc = ctx.enter_context(tile.TileContext(nc))
        const = ctx.enter_context(tc.tile_pool(name="const", bufs=1))
        hp = ctx.enter_context(tc.tile_pool(name="hp", bufs=4))
        work = ctx.enter_context(tc.tile_pool(name="work", bufs=4))
        ohp = ctx.enter_context(tc.tile_pool(name="ohp", bufs=4))
        outp = ctx.enter_context(tc.tile_pool(name="outp", bufs=4))
        psum = ctx.enter_context(tc.tile_pool(name="psum", bufs=1, space="PSUM"))

        # ---- load constants ----
        lat_sb = const.tile([128, BC], f32, tag="lat")
        nc.sync.dma_start(out=lat_sb[:], in_=lat[:])
        whh_sb = const.tile([128, 2, G3], bf16, tag="whh")
        nc.sync.dma_start(out=whh_sb[:], in_=whh[:])
        trz_sb = const.tile([KOH, 2, 512], fp8, tag="trz")
        nc.sync.dma_start(out=trz_sb[:], in_=trz[:])
        tnb_sb = const.tile([KOH, 2, 256], fp8, tag="tnb")
        nc.sync.dma_start(out=tnb_sb[:], in_=tnb[:])
        wd0_sb = const.tile([128, H], f32, tag="wd0")
        nc.sync.dma_start(out=wd0_sb[:], in_=wd0[:])
        wd1_sb = const.tile([128, 2, H], f32, tag="wd1")
        nc.sync.dma_start(out=wd1_sb[:], in_=wd1[:])
        wd2_sb = const.tile([128, 2, H], f32, tag="wd2")
        nc.sync.dma_start(out=wd2_sb[:], in_=wd2[:])
        wp1_sb = const.tile([128, 2, A], bf16, tag="wp1")
        nc.sync.dma_start(out=wp1_sb[:], in_=wp1[:])
        wp2_sb = const.tile([128, A], bf16, tag="wp2")
        nc.sync.dma_start(out=wp2_sb[:], in_=wp2[:])
        bp2_sb = const.tile([128, 4 * A], f32, tag="bp2")
        nc.sync.dma_start(
            out=bp2_sb[:],
            in_=bass.AP(tensor=bp2b, offset=0, ap=[[0, 128], [1, 4 * A]]),
        )
        bias_sb = const.tile([128, 9], f32, tag="bias")
        nc.sync.dma_start(out=bias_sb[:], in_=bias[:])


        # ---- MLP prologue: h0 = (relu(relu(lat@Wd0+b)@Wd1+b))@Wd2+b ----
        h1 = [work.tile([128, BC], f32, tag=f"mlp{m}", name=f"mlp{m}") for m in range(2)]
        for c in range(NCH):
            cs = slice(c * CH, (c + 1) * CH)
            ps = psum.tile([128, 2, CH], f32, tag="pr", bufs=2)
            for m in range(2):
                nc.tensor.matmul(
                    ps[:, m, :], wd0_sb[:, m * 128:(m + 1) * 128], lat_sb[:, cs],
                    start=True, stop=True,
                )
            for m in range(2):
                nc.vector.tensor_scalar(
                    out=h1[m][:, cs], in0=ps[:, m, :],
                    scalar1=bias_sb[:, 3:4] if m == 0 else bias_sb[:, 4:5],
                    scalar2=0.0, op0=Alu.add, op1=Alu.max,
                )
        h2 = [work.tile([128, BC], f32, tag=f"mlp2{m}", name=f"mlp2{m}") for m in range(2)]
        for c in range(NCH):
            cs = slice(c * CH, (c + 1) * CH)
            ps = psum.tile([128, 2, CH], f32, tag="pnz", bufs=2)
            for m in range(2):
                for kc in range(2):
                    nc.tensor.matmul(
                        ps[:, m, :], wd1_sb[:, kc, m * 128:(m + 1) * 128],
                        h1[kc][:, cs], start=(kc == 0), stop=(kc == 1),
                    )
            for m in range(2):
                nc.vector.tensor_scalar(
                    out=h2[m][:, cs], in0=ps[:, m, :],
                    scalar1=bias_sb[:, 5:6] if m == 0 else bias_sb[:, 6:7],
                    scalar2=0.0, op0=Alu.add, op1=Alu.max,
                )
        h_cur = hp.tile([128, 2, BC], bf16, tag="h", name="h0")
        for c in range(NCH):
            cs = slice(c * CH, (c + 1) * CH)
            ps = psum.tile([128, 2, CH], f32, tag="pr", bufs=2)
            for m in range(2):
                for kc in range(2):
                    nc.tensor.matmul(
                        ps[:, m, :], wd2_sb[:, kc, m * 128:(m + 1) * 128],
                        h2[kc][:, cs], start=(kc == 0), stop=(kc == 1),
                    )
            for m in range(2):
                nc.vector.tensor_scalar_add(
                    out=h_cur[:, m, cs], in0=ps[:, m, :],
                    scalar1=bias_sb[:, 7:8] if m == 0 else bias_sb[:, 8:9],
                )

        # ---- GRU steps ----
        def emit_proj_p1(h_tiles, tp):
            """p1 matmuls + relu for step tp (emitted after this step's
            sigmoid-r so the relus fill the ACT gap instead of blocking)."""
            p1ts = []
            for c in range(NCH):
                cs = slice(c * CH, (c + 1) * CH)
                p1ps = psum.tile([A, CH], f32, tag="pr", bufs=2,
                                 name=f"p1ps_{tp}_{c}")
                for kc in range(2):
                    nc.tensor.matmul(
                        p1ps[:], wp1_sb[:, kc, :], h_tiles[:, kc, cs],
                        start=(kc == 0), stop=(kc == 1),
                    )
                p1t = work.tile([A, CH], bf16, tag="p1t",
                                name=f"p1t_{tp}_{c}", bufs=2)
                nc.scalar.activation(
                    out=p1t[:], in_=p1ps[:], func=Act.Relu,
                    bias=bias_sb[0:A, 2:3],
                )
                p1ts.append(p1t)
            return p1ts

        def emit_proj_p2(p1ts, tp):
            for c in range(NCH):
                p2ps = psum.tile([128, 4 * A], f32, tag="pnz", bufs=2,
                                 name=f"p2ps_{tp}_{c}")
                for j in range(4):
                    nc.tensor.matmul(
                        p2ps[:, j * A:(j + 1) * A],
                        p1ts[c][:, j * 128:(j + 1) * 128], wp2_sb[0:A, :],
                        start=True, stop=True,
                    )
                outsb = outp.tile([128, 4 * A], f32, tag="outsb",
                                  name=f"outsb_{tp}_{c}")
                nc.vector.tensor_add(outsb[:], p2ps[:], bp2_sb[:])
                nc.sync.dma_start(
                    out=outv[c][:, :, tp, :],
                    in_=outsb.rearrange("p (j a) -> p j a", j=4),
                )

        def emit_proj(h_tiles, tp):
            emit_proj_p2(emit_proj_p1(h_tiles, tp), tp)

        h_prev = None
        for t in range(nsteps):
            oh_t = ohp.tile([KOH, 2, BC], fp8, tag="oh", name=f"oh_{t}")
            nc.sync.dma_start(out=oh_t[:], in_=oh[t])
            # n-gate input side (x16, incl b_ih) gathered by token on GPSIMD
            gin = work.tile([128, 2, BC], bf16, tag="gin", name=f"gin_{t}")
            if "gather" not in skip:
                for m in range(2):
                    nc.gpsimd.indirect_copy(
                        out=gin[:, m, :], data=gtab_sb[:, m, :],
                        idxs=tokw_sb[:, t, :],
                        i_know_ap_gather_is_preferred=True,
                    )
            else:
                nc.vector.memset(gin[:], 0.0)

            h_new = hp.tile([128, 2, BC], bf16, tag="h", name=f"h_{t}")
            pr_t, pn_t, pz_t = [], [], []
            # -- PE phase 1: r pre-acts (critical path head), then pn
            for c in range(NCH):
                cs = slice(c * CH, (c + 1) * CH)
                pr = psum.tile([128, 2, CH], f32, tag="pr", bufs=2,
                               name=f"pr_{t}_{c}")
                pr_t.append(pr)
                if "dr" not in skip:
                    for m in range(2):
                        nc.tensor.matmul(
                            pr[:, m, :], trz_sb[:, :, m * 128:(m + 1) * 128],
                            oh_t[:, :, cs], start=True, stop=False, perf_mode=DR,
                        )
                for kc in range(2):
                    for m in range(2):
                        nc.tensor.matmul(
                            pr[:, m, :], whh_sb[:, kc, m * 128:(m + 1) * 128],
                            h_cur[:, kc, cs],
                            start=(kc == 0 and "dr" in skip), stop=(kc == 1),
                        )
            for c in range(NCH):
                cs = slice(c * CH, (c + 1) * CH)
                pn = psum.tile([128, 2, CH], f32, tag="pnz", bufs=2,
                               name=f"pn_{t}_{c}")
                pn_t.append(pn)
                if "dr" not in skip:
                    for m in range(2):
                        nc.tensor.matmul(
                            pn[:, m, :], tnb_sb[:, :, m * 128:(m + 1) * 128],
                            oh_t[:, :, cs], start=True, stop=False, perf_mode=DR,
                        )
                for kc in range(2):
                    for m in range(2):
                        nc.tensor.matmul(
                            pn[:, m, :],
                            whh_sb[:, kc, 512 + m * 128:512 + (m + 1) * 128],
                            h_cur[:, kc, cs],
                            start=(kc == 0 and "dr" in skip), stop=(kc == 1),
                        )
            # -- projections of the previous step (optionally here)
            if (h_prev is not None and "proj" not in skip
                    and not opt.get("proj_late", True) and not opt.get("split_proj", True)):
                emit_proj(h_prev, t - 1)
            # -- PE phase 2: z pre-acts (reuse pr/pnz rotation slots)
            for c in range(NCH):
                cs = slice(c * CH, (c + 1) * CH)
                pz = psum.tile([128, 2, CH], f32,
                               tag="pr" if opt.get("pz_in_pr", True) else "pnz",
                               bufs=2, name=f"pz_{t}_{c}")
                pz_t.append(pz)
                if "dr" not in skip:
                    for m in range(2):
                        nc.tensor.matmul(
                            pz[:, m, :], trz_sb[:, :, 256 + m * 128:256 + (m + 1) * 128],
                            oh_t[:, :, cs], start=True, stop=False, perf_mode=DR,
                        )
                for kc in range(2):
                    for m in range(2):
                        nc.tensor.matmul(
                            pz[:, m, :],
                            whh_sb[:, kc, 256 + m * 128:256 + (m + 1) * 128],
                            h_cur[:, kc, cs],
                            start=(kc == 0 and "dr" in skip), stop=(kc == 1),
                        )

            rsb = [work.tile([128, 2, CH], bf16, tag=f"rsb{c}", name=f"rsb_{t}_{c}")
                   for c in range(NCH)]
            zsb = [work.tile([128, 2, CH], bf16, tag=f"zsb{c}", name=f"zsb_{t}_{c}")
                   for c in range(NCH)]
            npre = [work.tile([128, 2, CH], bf16, tag=f"np{c}", name=f"np_{t}_{c}")
                    for c in range(NCH)]
            t2 = [work.tile([128, 2, CH], bf16, tag=f"t2{c}", name=f"t2_{t}_{c}")
                  for c in range(NCH)]
            nsb = [work.tile([128, 2, CH], bf16, tag=f"n{c}", name=f"n_{t}_{c}")
                   for c in range(NCH)]
            dt_ = [work.tile([128, 2, CH], bf16, tag=f"d{c}", name=f"d_{t}_{c}")
                   for c in range(NCH)]

            # ACT: sigmoids/tanh, 1/S input scale folds out the x16
            for c in range(NCH):
                nc.scalar.activation(out=rsb[c][:], in_=pr_t[c][:],
                                     func=Act.Sigmoid, scale=1.0 / S)
            p1ts = None
            if _split and h_prev is not None and "proj" not in skip:
                p1ts = emit_proj_p1(h_prev, t - 1)
            # npre = pn * r on DVE (GPSIMD cannot access PSUM)
            for c in range(NCH):
                cs = slice(c * CH, (c + 1) * CH)
                nc.vector.tensor_mul(npre[c][:], pn_t[c][:], rsb[c][:])
                if c == 0 or not opt.get("gps_t2", False):
                    nc.vector.tensor_add(t2[c][:], npre[c][:], gin[:, :, cs])
                else:
                    nc.gpsimd.tensor_add(t2[c][:], npre[c][:], gin[:, :, cs])
                if opt.get("z_first", False):
                    nc.scalar.activation(out=zsb[c][:], in_=pz_t[c][:],
                                         func=Act.Sigmoid, scale=1.0 / S)
                    nc.scalar.activation(out=nsb[c][:], in_=t2[c][:],
                                         func=Act.Tanh, scale=1.0 / S)
                else:
                    nc.scalar.activation(out=nsb[c][:], in_=t2[c][:],
                                         func=Act.Tanh, scale=1.0 / S)
                    nc.scalar.activation(out=zsb[c][:], in_=pz_t[c][:],
                                         func=Act.Sigmoid, scale=1.0 / S)
            # combine h' = n + z*(h-n)
            for c in range(NCH):
                cs = slice(c * CH, (c + 1) * CH)
                nc.vector.tensor_sub(dt_[c][:], h_cur[:, :, cs], nsb[c][:])
                if c == 0 or not opt.get("gps_e", False):
                    nc.vector.tensor_mul(dt_[c][:], zsb[c][:], dt_[c][:])
                else:
                    nc.gpsimd.tensor_mul(dt_[c][:], zsb[c][:], dt_[c][:])
                nc.vector.tensor_add(h_new[:, :, cs], nsb[c][:], dt_[c][:])

            if _split:
                if p1ts is not None:
                    emit_proj_p2(p1ts, t - 1)
            elif h_prev is not None and "proj" not in skip and opt.get("proj_late", True):
                emit_proj(h_prev, t - 1)
            h_prev = h_new
            h_cur = h_new
        if "proj" not in skip:
            emit_proj(h_prev, T - 1)

    nc.finalize()
    return nc


def _prep_inputs(latent, target, embed, W_ih, b_ih, W_hh, b_hh,
                 Wd0, bd0, Wd1, bd1, Wd2, bd2, Wp1, bp1, Wp2, bp2):
    f32 = np.float32
    latent = np.asarray(latent, dtype=f32)
    embed = np.asarray(embed, dtype=f32)
    W_ih = np.asarray(W_ih, dtype=f32)
    b_ih = np.asarray(b_ih, dtype=f32)
    W_hh = np.asarray(W_hh, dtype=f32)
    b_hh = np.asarray(b_hh, dtype=f32)

    tokens = np.concatenate(
        [np.zeros((B, 1), dtype=np.int64), np.asarray(target[:, :-1], dtype=np.int64)],
        axis=1,
    )  # [B, T]

    # one-hot rows 0..31 + const-1 row 32, DoubleRow layout k=(j*KOH+p)
    ohf = np.zeros((T, 2 * KOH, B), dtype=FP8)
    tok_tm = tokens.T
    for a in range(A):
        ohf[:, a, :] = (tok_tm == a)
    ohf[:, A, :] = 1.0
    ohdr = ohf.reshape(T, 2, KOH, B).transpose(0, 2, 1, 3)  # [T, KOH, 2, B]
    ohdr = np.ascontiguousarray(ohdr)

    giv = embed @ W_ih.T  # [A, 768]
    # r,z input table (+ all r/z biases on const row), x S, fp8
    tabrz_rows = np.zeros((2 * KOH, 512), dtype=f32)
    tabrz_rows[:A, :] = S * giv[:, :2 * H]
    tabrz_rows[A, :] = S * (b_ih + b_hh)[:2 * H]
    trz = np.ascontiguousarray(
        tabrz_rows.reshape(2, KOH, 512).transpose(1, 0, 2)).astype(FP8)
    # n bias table: const row -> S*b_hh_n
    tabnb_rows = np.zeros((2 * KOH, 256), dtype=f32)
    tabnb_rows[A, :] = S * b_hh[2 * H:]
    tnb = np.ascontiguousarray(
        tabnb_rows.reshape(2, KOH, 256).transpose(1, 0, 2)).astype(FP8)

    # n input side S*(giv_n + b_ih_n) pre-gathered by token for all steps:
    # [T, 128, 2, B] (sliced per core below)
    givT_n = S * (giv.T[2 * H:] + b_ih[2 * H:, None])     # [256, 32]
    gtab_l = givT_n.reshape(2, 128, A).transpose(1, 0, 2).astype(BF16)  # [128,2,A]
    ging_all = np.ascontiguousarray(
        gtab_l[:, :, tokens.T]).transpose(2, 0, 1, 3)     # [T, 128, 2, B]

    whhT = np.ascontiguousarray(S * W_hh.T)               # [H, 3H] x S
    whh_l = np.ascontiguousarray(
        whhT.reshape(2, 128, G3).transpose(1, 0, 2)).astype(BF16)

    wd0_l = np.ascontiguousarray(np.asarray(Wd0, dtype=f32))
    wd1_l = np.ascontiguousarray(
        np.asarray(Wd1, dtype=f32).reshape(2, 128, H).transpose(1, 0, 2))
    wd2_l = np.ascontiguousarray(
        np.asarray(Wd2, dtype=f32).reshape(2, 128, H).transpose(1, 0, 2))
    wp1_l = np.ascontiguousarray(
        np.asarray(Wp1, dtype=f32).reshape(2, 128, A).transpose(1, 0, 2)).astype(BF16)
    wp2_l = np.ascontiguousarray(
        np.tile(np.asarray(Wp2, dtype=f32), (4, 1))).astype(BF16)
    bp2b = np.ascontiguousarray(
        np.tile(np.asarray(bp2, dtype=f32), 4)[None, :])

    bias_pack = np.zeros((128, 9), dtype=f32)
    bias_pack[:A, 2] = np.asarray(bp1, dtype=f32)
    bias_pack[:, 3] = np.asarray(bd0, dtype=f32)[:128]
    bias_pack[:, 4] = np.asarray(bd0, dtype=f32)[128:]
    bias_pack[:, 5] = np.asarray(bd1, dtype=f32)[:128]
    bias_pack[:, 6] = np.asarray(bd1, dtype=f32)[128:]
    bias_pack[:, 7] = np.asarray(bd2, dtype=f32)[:128]
    bias_pack[:, 8] = np.asarray(bd2, dtype=f32)[128:]

    latT = np.ascontiguousarray(latent.T)  # [128, B]

    shared = dict(whh=whh_l, trz=trz, tnb=tnb, wd0=wd0_l,
                  wd1=wd1_l, wd2=wd2_l, wp1=wp1_l, wp2=wp2_l, bp2b=bp2b, bias=bias_pack)
    in_maps = []
    for c in range(NCORES):
        bs = slice(c * BC, (c + 1) * BC)
        m = dict(shared)
        m["lat"] = np.ascontiguousarray(latT[:, bs])
        m["oh"] = np.ascontiguousarray(ohdr[:, :, :, bs])
        m["ging"] = np.ascontiguousarray(ging_all[:, :, :, bs])
        in_maps.append(m)
    return in_maps


def kernel(**inputs):
    from concourse.bass_utils import run_bass_kernel_spmd

    if "nc" not in _CACHE:
        _CACHE["nc"] = _build()
    nc = _CACHE["nc"]

    in_maps = _prep_inputs(**inputs)
    res = run_bass_kernel_spmd(nc, in_maps, core_ids=list(range(NCORES)))
    outs = [r["out"] for r in res.results]
    return np.concatenate(outs, axis=0).astype(np.float32)
